# revision 8
# baseline (speedup 1.0000x reference)
"""TRN2 Bass kernel for nn_Model_14731737825320 (change-point grouped-channel
transformer). Self-contained: hardcodes shapes/sharding.

Strategy (8 NeuronCores, SPMD):
  Phase 1 (change point): data-parallel over batch, core b handles batch b.
    Computes global embedding, time-attention scores, and per-row softmax
    diagonal contributions; host reduces 8x[512] -> diag, argmax(|diff|) -> cp.
  Phase 2 (grouped encoder): core c = g*4+i handles direction-group g,
    batches {2i, 2i+1}. Dynamic change-point handled with host-built
    shift-select matrices / segment positional table / additive attention
    mask fed as inputs, so the compiled program is cp-independent.
    Only the last PRED=96 query positions are computed through the encoder
    block (attention context K/V uses all 512 positions).
  Host: assembles [8,96,32] output (each group's 16 channels share a plane).
"""
import os

os.environ.setdefault("MYCRO_LOCAL_CACHE", "1")

import numpy as np

import concourse.bass as bass
from concourse import bacc
import concourse.mybir as mybir
import concourse.tile as tile
from concourse.bass import ts
from concourse.bass_utils import run_bass_kernel_spmd

B, L, C, D, H, DFF, PRED = 8, 512, 32, 512, 8, 2048, 96
dk = D // H  # 64
P = 128
QS = L - PRED  # 416 first tail query
F32 = mybir.dt.float32
I32 = mybir.dt.int32
AX = mybir.AxisListType.X
AF = mybir.ActivationFunctionType
TAIL = slice(QS, L)


def _sinusoid(n, d):
    pos = np.arange(n, dtype=np.float32)[:, None]
    div = np.exp(np.arange(0, d, 2, dtype=np.float32) * (-np.log(10000.0) / d))
    pe = np.zeros((n, d), np.float32)
    pe[:, 0::2] = np.sin(pos * div)
    pe[:, 1::2] = np.cos(pos * div)
    return pe


POS = _sinusoid(L, D)
TT7 = np.concatenate([_sinusoid(n, D)[:7] for n in (13, 32, 7, 24)], 0)  # [28,D]
SEL4 = np.zeros((4, 28), np.float32)
for _i in range(4):
    SEL4[_i, 7 * _i:7 * _i + 7] = 1.0
IOTA28 = (np.arange(28, dtype=np.float32) % 7)[:, None]  # [28,1]
IDENT = np.eye(P, dtype=np.float32)
ONES1 = np.ones((1, L), np.float32)


# ---------------------------------------------------------------- shared emit
def _emit_onehot(nc, pools, xm_dram_2d, cst):
    """x_mark [L,4] i32 -> onehotT [28, L] f32 (row 7i+r == 1 if mark[:,i]==r)."""
    sb, pss = pools["sb"], pools["pss"]
    xmi = sb.tile([4, L], I32, tag="xmi")
    nc.sync.dma_start(xmi[:], xm_dram_2d.rearrange("l i -> i l"))
    xmf = sb.tile([4, L], F32, tag="xmf")
    nc.vector.tensor_copy(xmf[:], xmi[:])
    rep = pss.tile([28, L], F32, tag="psS")
    nc.tensor.matmul(rep[:], lhsT=cst["sel4"][:], rhs=xmf[:], start=True, stop=True)
    oh = sb.tile([28, L], F32, tag="onehot")
    nc.vector.tensor_scalar(oh[:], rep[:], cst["iota28"][:], None,
                            mybir.AluOpType.is_equal)
    return oh


def _load_w_kpo(nc, pool, dram_ap, kdim, n, tag):
    """[kdim, n] DRAM -> sbuf [128, kdim//128, n]."""
    t = pool.tile([P, kdim // P, n], F32, tag=tag, name=tag)
    nc.sync.dma_start(t[:], dram_ap.rearrange("(o p) n -> p o n", p=P))
    return t


def _load_vec_po(nc, pool, dram_ap, n, tag):
    """[n] DRAM -> sbuf [128, n//128]  (chunk c = column c)."""
    t = pool.tile([P, n // P], F32, tag=tag, name=tag)
    nc.sync.dma_start(t[:], dram_ap.rearrange("(o p) -> p o", p=P))
    return t


def _ln(nc, pools, out, x, gb, bb):
    """layernorm over free dim (512) of token-major [96, 512].
    Clobbers x (uses it as squared-deviation scratch)."""
    sb = pools["sb"]
    mu = sb.tile([PRED, 1], F32, tag="ln_mu")
    nc.vector.reduce_sum(mu[:], x[:], axis=AX)
    nc.vector.tensor_scalar_mul(mu[:], mu[:], 1.0 / D)
    nc.vector.tensor_scalar(out[:], x[:], mu[:], None, mybir.AluOpType.subtract)
    nc.vector.tensor_mul(x[:], out[:], out[:])
    vs = sb.tile([PRED, 1], F32, tag="ln_vs")
    nc.vector.reduce_sum(vs[:], x[:], axis=AX)
    nc.vector.tensor_scalar(vs[:], vs[:], 1.0 / D, 1e-5,
                            mybir.AluOpType.mult, mybir.AluOpType.add)
    sd = sb.tile([PRED, 1], F32, tag="ln_sd")
    nc.scalar.activation(sd[:], vs[:], AF.Sqrt)
    isd = sb.tile([PRED, 1], F32, tag="ln_isd")
    nc.vector.reciprocal(isd[:], sd[:])
    nc.vector.tensor_scalar_mul(out[:], out[:], isd[:])
    nc.vector.tensor_mul(out[:], out[:], gb[:PRED, :])
    nc.vector.tensor_add(out[:], out[:], bb[:PRED, :])


def _bcast_row(nc, pools, row_sb, tag):
    """[1,512] sbuf -> [128,512] (outer product with ones via PE)."""
    ps = pools["psb"].tile([P, D], F32, tag="psB", name="ps_bc")
    nc.tensor.matmul(ps[:], lhsT=pools["cst"]["ones1"][:1, :P], rhs=row_sb[:],
                     start=True, stop=True)
    t = pools["sb1"].tile([P, D], F32, tag=tag, name=tag)
    nc.vector.tensor_copy(t[:], ps[:])
    return t


# ---------------------------------------------------------------- phase 1
def build_phase1():
    nc = bacc.Bacc("TRN2", target_bir_lowering=False, debug=False)
    d_in = {}

    def inp(name, shape, dt=F32):
        d_in[name] = nc.dram_tensor(name, list(shape), dt, kind="ExternalInput").ap()
        return d_in[name]

    x = inp("x", (L, C))
    xm = inp("xm", (L, 4), I32)
    wq = inp("wq", (D, D)); wk = inp("wk", (D, D))
    bq = inp("bq", (D,)); bk = inp("bk", (D,))
    wcat = inp("wcat", (3 * C, D))
    tt7 = inp("tt7", (28, D)); sel4 = inp("sel4", (4, 28))
    iota28 = inp("iota28", (28, 1)); ident = inp("ident", (P, P))
    posT = inp("posT", (D, L)); ones1 = inp("ones1", (1, L))
    out = nc.dram_tensor("diag", [P, 4], F32, kind="ExternalOutput").ap()

    with tile.TileContext(nc) as tc:
        with tc.tile_pool(name="sb1", bufs=1) as sb1, \
             tc.tile_pool(name="sb", bufs=2) as sb, \
             tc.tile_pool(name="psb", bufs=3, space="PSUM") as psb, \
             tc.tile_pool(name="pss", bufs=3, space="PSUM") as pss:
            pools = dict(sb1=sb1, sb=sb, psb=psb, pss=pss)
            cst = {}
            for nm, ap_, sh in (("sel4", sel4, (4, 28)), ("iota28", iota28, (28, 1)),
                                ("ident", ident, (P, P)), ("tt7", tt7, (28, D)),
                                ("wcat", wcat, (3 * C, D)), ("ones1", ones1, (1, L))):
                cst[nm] = sb1.tile(list(sh), F32, tag="c_" + nm, name="c_" + nm)
                nc.sync.dma_start(cst[nm][:], ap_)
            pools["cst"] = cst
            wq_sb = _load_w_kpo(nc, sb1, wq, D, D, "wq")
            wk_sb = _load_w_kpo(nc, sb1, wk, D, D, "wk")
            posT_sb = _load_w_kpo(nc, sb1, posT, D, L, "posT")
            bq_sb = _load_vec_po(nc, sb1, bq, D, "bq")
            bk_sb = _load_vec_po(nc, sb1, bk, D, "bk")
            bq8 = sb1.tile([P, 4], F32, tag="bq8")
            nc.vector.tensor_scalar_mul(bq8[:], bq_sb[:], 0.125)

            # x -> xcatT [96, 512] : rows 0-31 prev, 32-63 x^T, 64-95 next
            x_sb = sb.tile([P, 4, C], F32, tag="x")
            nc.sync.dma_start(x_sb[:], x.rearrange("(o p) c -> p o c", p=P))
            xcatT = sb.tile([3 * C, L], F32, tag="xcatT")
            for o in range(4):
                pt = pss.tile([C, P], F32, tag="psS")
                nc.tensor.transpose(pt[:], x_sb[:, o, :], cst["ident"][:])
                nc.vector.tensor_copy(xcatT[C:2 * C, ts(o, P)], pt[:])
            nc.vector.tensor_copy(xcatT[0:C, 1:L], xcatT[C:2 * C, 0:L - 1])
            nc.vector.tensor_copy(xcatT[0:C, 0:1], xcatT[C:2 * C, L - 1:L])
            nc.vector.tensor_copy(xcatT[2 * C:3 * C, 0:L - 1], xcatT[C:2 * C, 1:L])
            nc.vector.tensor_copy(xcatT[2 * C:3 * C, L - 1:L], xcatT[C:2 * C, 0:1])

            oh = _emit_onehot(nc, pools, xm, cst)

            # geT [128, 4, 512]
            geT = sb1.tile([P, 4, L], F32, tag="geT")
            for c in range(4):
                ps = psb.tile([P, L], F32, tag="psB")
                nc.tensor.matmul(ps[:], lhsT=cst["wcat"][:, ts(c, P)], rhs=xcatT[:],
                                 start=True, stop=False)
                nc.tensor.matmul(ps[:], lhsT=cst["tt7"][:, ts(c, P)], rhs=oh[:],
                                 start=False, stop=True)
                nc.vector.tensor_add(geT[:, c, :], ps[:], posT_sb[:, c, :])

            # qT (scaled by 1/8), kT  [128, 4, 512]
            qT = sb1.tile([P, 4, L], F32, tag="qT")
            kT = sb1.tile([P, 4, L], F32, tag="kT")
            for m in range(4):
                psq = psb.tile([P, L], F32, tag="psB")
                psk = psb.tile([P, L], F32, tag="psB")
                for k in range(4):
                    nc.tensor.matmul(psq[:], lhsT=wq_sb[:, k, ts(m, P)],
                                     rhs=geT[:, k, :], start=(k == 0), stop=(k == 3))
                for k in range(4):
                    nc.tensor.matmul(psk[:], lhsT=wk_sb[:, k, ts(m, P)],
                                     rhs=geT[:, k, :], start=(k == 0), stop=(k == 3))
                nc.scalar.activation(qT[:, m, :], psq[:], AF.Identity,
                                     bias=bq8[:, m:m + 1], scale=0.125)
                nc.scalar.activation(kT[:, m, :], psk[:], AF.Identity,
                                     bias=bk_sb[:, m:m + 1], scale=1.0)

            # scores / exp / diag accumulation
            acc = sb1.tile([P, 4], F32, tag="acc")
            nc.vector.memset(acc[:], 0.0)
            for h in range(H):
                hk, off = h // 2, 64 * (h % 2)
                for qc in range(4):
                    pssc = psb.tile([P, L], F32, tag="psB")
                    nc.tensor.matmul(pssc[:],
                                     lhsT=qT[off:off + dk, hk, ts(qc, P)],
                                     rhs=kT[off:off + dk, hk, :],
                                     start=True, stop=True)
                    es = sb.tile([P, L], F32, tag="expS")
                    r = sb.tile([P, 1], F32, tag="r")
                    nc.scalar.activation(es[:], pssc[:], AF.Exp, accum_out=r[:])
                    rr = sb.tile([P, 1], F32, tag="rr")
                    nc.vector.reciprocal(rr[:], r[:])
                    dtmp = sb.tile([P, P], F32, tag="dtmp")
                    nc.vector.tensor_mul(dtmp[:], es[:, ts(qc, P)], cst["ident"][:])
                    dg = sb.tile([P, 1], F32, tag="dg")
                    nc.vector.reduce_sum(dg[:], dtmp[:], axis=AX)
                    nc.vector.tensor_mul(dg[:], dg[:], rr[:])
                    nc.vector.tensor_add(acc[:, qc:qc + 1], acc[:, qc:qc + 1], dg[:])
            nc.sync.dma_start(out, acc[:])
    nc.compile()
    return nc


# ---------------------------------------------------------------- phase 2
def build_phase2():
    nc = bacc.Bacc("TRN2", target_bir_lowering=False, debug=False)

    def inp(name, shape, dt=F32):
        return nc.dram_tensor(name, list(shape), dt, kind="ExternalInput").ap()

    x2 = inp("x2", (2, L, 16))
    xm2 = inp("xm2", (2, L, 4), I32)
    wq = inp("wq", (D, D)); wk = inp("wk", (D, D))
    wv = inp("wv", (D, D)); wo = inp("wo", (D, D))
    bq = inp("bq", (D,)); bk = inp("bk", (D,))
    bv = inp("bv", (1, D)); bo = inp("bo", (1, D))
    wcatg = inp("wcatg", (80, D))
    w1 = inp("w1", (D, DFF)); b1m = inp("b1m", (DFF,))
    w2 = inp("w2", (DFF, D)); b2m = inp("b2m", (1, D))
    g1 = inp("g1", (1, D)); b1n = inp("b1n", (1, D))
    g2 = inp("g2", (1, D)); b2n = inp("b2n", (1, D))
    pw = inp("pw", (D, 1)); pb = inp("pb", (1, 1))
    tt7 = inp("tt7", (28, D)); sel4 = inp("sel4", (4, 28))
    iota28 = inp("iota28", (28, 1)); ident = inp("ident", (P, P))
    possegT = inp("possegT", (D, L)); ones1 = inp("ones1", (1, L))
    selLT = inp("selLT", (L, L)); selRT = inp("selRT", (L, L))
    maskT = inp("maskT", (L, PRED))
    out = nc.dram_tensor("y", [PRED, 2], F32, kind="ExternalOutput").ap()

    with tile.TileContext(nc) as tc:
        with tc.tile_pool(name="sb1", bufs=1) as sb1, \
             tc.tile_pool(name="sbB", bufs=1) as sbB, \
             tc.tile_pool(name="sb", bufs=1) as sb, \
             tc.tile_pool(name="sbe", bufs=2) as sbe, \
             tc.tile_pool(name="psb", bufs=3, space="PSUM") as psb, \
             tc.tile_pool(name="pss", bufs=3, space="PSUM") as pss, \
             tc.tile_pool(name="po", bufs=1, space="PSUM") as po, \
             tc.tile_pool(name="pr", bufs=1, space="PSUM") as pr:
            pools = dict(sb1=sb1, sb=sb, psb=psb, pss=pss)
            cst = {}
            for nm, ap_, sh in (("sel4", sel4, (4, 28)), ("iota28", iota28, (28, 1)),
                                ("ident", ident, (P, P)), ("tt7", tt7, (28, D)),
                                ("wcatg", wcatg, (80, D)), ("ones1", ones1, (1, L)),
                                ("bv", bv, (1, D)), ("bo", bo, (1, D)),
                                ("b2m", b2m, (1, D)), ("pb", pb, (1, 1))):
                cst[nm] = sb1.tile(list(sh), F32, tag="c_" + nm, name="c_" + nm)
                nc.sync.dma_start(cst[nm][:], ap_)
            pools["cst"] = cst
            pools["psb"] = psb

            wq_sb = _load_w_kpo(nc, sb1, wq, D, D, "wq")
            wk_sb = _load_w_kpo(nc, sb1, wk, D, D, "wk")
            wv_sb = _load_w_kpo(nc, sb1, wv, D, D, "wv")
            wo_sb = _load_w_kpo(nc, sb1, wo, D, D, "wo")
            w1_sb = _load_w_kpo(nc, sb1, w1, D, DFF, "w1")
            w2_sb = _load_w_kpo(nc, sb1, w2, DFF, D, "w2")
            posT_sb = _load_w_kpo(nc, sb1, possegT, D, L, "posT")
            selL_sb = _load_w_kpo(nc, sb1, selLT, L, L, "selL")
            selR_sb = _load_w_kpo(nc, sb1, selRT, L, L, "selR")
            maskT_sb = _load_w_kpo(nc, sb1, maskT, L, PRED, "maskT")
            bq_sb = _load_vec_po(nc, sb1, bq, D, "bq")
            bk_sb = _load_vec_po(nc, sb1, bk, D, "bk")
            b1_sb = _load_vec_po(nc, sb1, b1m, DFF, "b1m")
            pw_sb = sb1.tile([P, 4], F32, tag="pw")
            nc.sync.dma_start(pw_sb[:], pw.rearrange("(o p) one -> p (o one)", p=P))
            bq8 = sb1.tile([P, 4], F32, tag="bq8")
            nc.vector.tensor_scalar_mul(bq8[:], bq_sb[:], 0.125)
            ones_c = sb1.tile([P, 1], F32, tag="ones_c")
            nc.vector.memset(ones_c[:], 1.0)

            # broadcast norm params to [128, 512]
            gb1 = gb2 = bb1 = bb2 = None
            rows = {}
            for nm, ap_ in (("g1", g1), ("b1n", b1n), ("g2", g2), ("b2n", b2n)):
                r_ = sb1.tile([1, D], F32, tag="row_" + nm, name="row_" + nm)
                nc.sync.dma_start(r_[:], ap_)
                rows[nm] = _bcast_row(nc, pools, r_, "bc_" + nm)
            gb1, bb1, gb2, bb2 = rows["g1"], rows["b1n"], rows["g2"], rows["b2n"]

            y_sb = sb1.tile([PRED, 2], F32, tag="y")

            for b in range(2):
                # ---- embedding: xcatT [48, 512]
                x_sb = sb.tile([P, 4, 16], F32, tag="x")
                nc.sync.dma_start(x_sb[:], x2[b].rearrange("(o p) c -> p o c", p=P))
                xcatT = sb.tile([80, L], F32, tag="xcatT")
                nc.vector.memset(xcatT[:], 0.0)
                psl = pss.tile([16, L], F32, tag="psS")
                psr = pss.tile([16, L], F32, tag="psS")
                for k in range(4):
                    nc.tensor.matmul(psl[:], lhsT=x_sb[:, k, :], rhs=selL_sb[:, k, :],
                                     start=(k == 0), stop=(k == 3))
                for k in range(4):
                    nc.tensor.matmul(psr[:], lhsT=x_sb[:, k, :], rhs=selR_sb[:, k, :],
                                     start=(k == 0), stop=(k == 3))
                nc.vector.tensor_copy(xcatT[0:16, :], psl[:])
                nc.vector.tensor_copy(xcatT[64:80, :], psr[:])
                for o in range(4):
                    pt = pss.tile([16, P], F32, tag="psS")
                    nc.tensor.transpose(pt[:], x_sb[:, o, :], cst["ident"][:])
                    nc.vector.tensor_copy(xcatT[32:48, ts(o, P)], pt[:])

                oh = _emit_onehot(nc, pools, xm2[b], cst)

                geT = sbB.tile([P, 4, L], F32, tag="geT")
                for c in range(4):
                    ps = psb.tile([P, L], F32, tag="psB")
                    nc.tensor.matmul(ps[:], lhsT=cst["wcatg"][:, ts(c, P)],
                                     rhs=xcatT[:], start=True, stop=False)
                    nc.tensor.matmul(ps[:], lhsT=cst["tt7"][:, ts(c, P)], rhs=oh[:],
                                     start=False, stop=True)
                    nc.vector.tensor_add(geT[:, c, :], ps[:], posT_sb[:, c, :])

                # ---- K^T, V(token-major), Q^T(tail)
                kT = sbB.tile([P, 4, L], F32, tag="kT")
                qT = sbB.tile([P, 4, PRED], F32, tag="qT")
                V = sbB.tile([P, 4, D], F32, tag="V")
                for m in range(4):
                    psk = psb.tile([P, L], F32, tag="psB")
                    for k in range(4):
                        nc.tensor.matmul(psk[:], lhsT=wk_sb[:, k, ts(m, P)],
                                         rhs=geT[:, k, :], start=(k == 0),
                                         stop=(k == 3))
                    nc.scalar.activation(kT[:, m, :], psk[:], AF.Identity,
                                         bias=bk_sb[:, m:m + 1], scale=1.0)
                    psq = pss.tile([P, PRED], F32, tag="psS")
                    for k in range(4):
                        nc.tensor.matmul(psq[:], lhsT=wq_sb[:, k, ts(m, P)],
                                         rhs=geT[:, k, TAIL], start=(k == 0),
                                         stop=(k == 3))
                    nc.scalar.activation(qT[:, m, :], psq[:], AF.Identity,
                                         bias=bq8[:, m:m + 1], scale=0.125)
                    psv = psb.tile([P, D], F32, tag="psB")
                    for k in range(4):
                        nc.tensor.matmul(psv[:], lhsT=geT[:, k, ts(m, P)],
                                         rhs=wv_sb[:, k, :], start=(k == 0),
                                         stop=False)
                    nc.tensor.matmul(psv[:], lhsT=cst["ones1"][:1, ts(m, P)],
                                     rhs=cst["bv"][:], start=False, stop=True)
                    nc.vector.tensor_copy(V[:, m, :], psv[:])

                # ---- attention (tail queries)
                psO = po.tile([PRED, D], F32, tag="psO")
                psR = pr.tile([PRED, H], F32, tag="psR")
                for h in range(H):
                    hk, off = h // 2, 64 * (h % 2)
                    est = sbe.tile([P, 4, PRED], F32, tag="est")
                    for c in range(4):
                        pst = pss.tile([P, PRED], F32, tag="psS")
                        nc.tensor.matmul(pst[:],
                                         lhsT=kT[off:off + dk, hk, ts(c, P)],
                                         rhs=qT[off:off + dk, hk, :],
                                         start=True, stop=True)
                        nc.vector.tensor_add(est[:, c, :], pst[:],
                                             maskT_sb[:, c, :])
                        nc.scalar.activation(est[:, c, :], est[:, c, :], AF.Exp)
                    for c in range(4):
                        nc.tensor.matmul(psO[:, ts(h, dk)], lhsT=est[:, c, :],
                                         rhs=V[:, c, ts(h, dk)], start=(c == 0),
                                         stop=(c == 3), skip_group_check=True)
                        nc.tensor.matmul(psR[:, h:h + 1], lhsT=est[:, c, :],
                                         rhs=ones_c[:], start=(c == 0),
                                         stop=(c == 3), skip_group_check=True)
                rrec = sb.tile([PRED, H], F32, tag="rrec")
                nc.vector.reciprocal(rrec[:], psR[:])
                O_sb = sb.tile([PRED, D], F32, tag="O")
                for h in range(H):
                    nc.vector.tensor_scalar_mul(O_sb[:, ts(h, dk)],
                                                psO[:, ts(h, dk)],
                                                rrec[:, h:h + 1])

                # ---- out-proj + residual + LN1
                oT = sb.tile([P, 4, PRED], F32, tag="oT")
                for c in range(4):
                    pt = pss.tile([P, PRED], F32, tag="psS")
                    nc.tensor.transpose(pt[:], O_sb[:, ts(c, P)],
                                        cst["ident"][:PRED, :PRED])
                    nc.vector.tensor_copy(oT[:, c, :], pt[:])
                ps0 = psb.tile([PRED, D], F32, tag="psB")
                for c in range(4):
                    nc.tensor.matmul(ps0[:], lhsT=oT[:, c, :], rhs=wo_sb[:, c, :],
                                     start=(c == 0), stop=False)
                nc.tensor.matmul(ps0[:], lhsT=cst["ones1"][:1, :PRED],
                                 rhs=cst["bo"][:], start=False, stop=True)
                x0 = sb.tile([PRED, D], F32, tag="x0")
                for c in range(4):
                    pt = pss.tile([PRED, P], F32, tag="psS")
                    nc.tensor.transpose(pt[:], geT[:, c, TAIL], cst["ident"][:])
                    nc.vector.tensor_copy(x0[:, ts(c, P)], pt[:])
                nc.vector.tensor_add(x0[:], ps0[:], x0[:])
                x1 = sb.tile([PRED, D], F32, tag="x1")
                _ln(nc, pools, x1, x0, gb1, bb1)

                # ---- MLP
                x1T = sb.tile([P, 4, PRED], F32, tag="x1T")
                for c in range(4):
                    pt = pss.tile([P, PRED], F32, tag="psS")
                    nc.tensor.transpose(pt[:], x1[:, ts(c, P)],
                                        cst["ident"][:PRED, :PRED])
                    nc.vector.tensor_copy(x1T[:, c, :], pt[:])
                hT = sbB.tile([P, 16, PRED], F32, tag="hT")
                for f in range(16):
                    psh = pss.tile([P, PRED], F32, tag="psS")
                    for k in range(4):
                        nc.tensor.matmul(psh[:], lhsT=w1_sb[:, k, ts(f, P)],
                                         rhs=x1T[:, k, :], start=(k == 0),
                                         stop=(k == 3))
                    nc.scalar.activation(hT[:, f, :], psh[:], AF.Gelu,
                                         bias=b1_sb[:, f:f + 1], scale=1.0)
                psz = psb.tile([PRED, D], F32, tag="psB")
                for f in range(16):
                    nc.tensor.matmul(psz[:], lhsT=hT[:, f, :], rhs=w2_sb[:, f, :],
                                     start=(f == 0), stop=False)
                nc.tensor.matmul(psz[:], lhsT=cst["ones1"][:1, :PRED],
                                 rhs=cst["b2m"][:], start=False, stop=True)
                nc.vector.tensor_add(x1[:], psz[:], x1[:])
                x2t = sb.tile([PRED, D], F32, tag="x2t")
                _ln(nc, pools, x2t, x1, gb2, bb2)

                # ---- projection
                x2T = sb.tile([P, 4, PRED], F32, tag="x2T")
                for c in range(4):
                    pt = pss.tile([P, PRED], F32, tag="psS")
                    nc.tensor.transpose(pt[:], x2t[:, ts(c, P)],
                                        cst["ident"][:PRED, :PRED])
                    nc.vector.tensor_copy(x2T[:, c, :], pt[:])
                psy = pss.tile([PRED, 1], F32, tag="psS")
                for k in range(4):
                    nc.tensor.matmul(psy[:], lhsT=x2T[:, k, :],
                                     rhs=pw_sb[:, k:k + 1], start=(k == 0),
                                     stop=False)
                nc.tensor.matmul(psy[:], lhsT=cst["ones1"][:1, :PRED],
                                 rhs=cst["pb"][:], start=False, stop=True)
                nc.vector.tensor_copy(y_sb[:, b:b + 1], psy[:])

            nc.sync.dma_start(out, y_sb[:])
    nc.compile()
    return nc


# ---------------------------------------------------------------- host glue
_NC1 = None
_NC2 = None


def _get_programs():
    global _NC1, _NC2
    if _NC1 is None:
        _NC1 = build_phase1()
    if _NC2 is None:
        _NC2 = build_phase2()
    return _NC1, _NC2


def _np(a):
    return np.ascontiguousarray(np.asarray(a, dtype=np.float32))


def _pad_wcat(w3):
    """[3,16,D] -> [80,D]: rows 0:16=w[0], 32:48=w[1], 64:80=w[2], rest zero."""
    out = np.zeros((80, D), np.float32)
    out[0:16] = w3[0]
    out[32:48] = w3[1]
    out[64:80] = w3[2]
    return out


def kernel(x_enc, x_mark_enc, x_dec, x_mark_dec, params, _timing=None):
    nc1, nc2 = _get_programs()
    x_enc = np.asarray(x_enc, np.float32)
    xm = np.ascontiguousarray(np.asarray(x_mark_enc, np.int32))
    p = params
    ta = p["time_attn"]

    common1 = dict(wq=_np(ta["wq"]), wk=_np(ta["wk"]), bq=_np(ta["bq"]),
                   bk=_np(ta["bk"]), wcat=_np(p["glob_conv"]).reshape(3 * C, D),
                   tt7=TT7, sel4=SEL4, iota28=IOTA28, ident=IDENT,
                   posT=np.ascontiguousarray(POS.T), ones1=ONES1)
    in_maps1 = [dict(x=np.ascontiguousarray(x_enc[b]), xm=xm[b], **common1)
                for b in range(B)]
    res1 = run_bass_kernel_spmd(nc1, in_maps1, core_ids=list(range(8)))
    if _timing is not None:
        _timing.append(res1)
    acc = np.zeros(L, np.float32)
    for b in range(B):
        acc += res1.results[b]["diag"].T.reshape(L)
    diag = acc / (B * H)
    d = np.abs(np.diff(diag))
    cp = int(np.argsort(d)[-1]) + 1

    # cp-derived host tensors
    seg = (np.arange(L) >= cp)
    maskT = np.where(seg[:, None] != seg[None, QS:], np.float32(-1e9),
                     np.float32(0)).astype(np.float32)
    posseg = np.concatenate([POS[:cp], POS[:L - cp]], 0)
    lm1 = np.array([cp - 1 if l == 0 else (L - 1 if l == cp else l - 1)
                    for l in range(L)])
    lp1 = np.array([0 if l == cp - 1 else (cp if l == L - 1 else l + 1)
                    for l in range(L)])
    selLT = np.zeros((L, L), np.float32)
    selLT[lm1, np.arange(L)] = 1.0  # [l_src, j]
    selRT = np.zeros((L, L), np.float32)
    selRT[lp1, np.arange(L)] = 1.0

    common2 = dict(w1=_np(p["mlp_w1"]), b1m=_np(p["mlp_b1"]), w2=_np(p["mlp_w2"]),
                   b2m=_np(p["mlp_b2"]).reshape(1, D),
                   g1=_np(p["norm1_g"]).reshape(1, D),
                   b1n=_np(p["norm1_b"]).reshape(1, D),
                   g2=_np(p["norm2_g"]).reshape(1, D),
                   b2n=_np(p["norm2_b"]).reshape(1, D),
                   pw=_np(p["proj_w"]), pb=_np(p["proj_b"]).reshape(1, 1),
                   tt7=TT7, sel4=SEL4, iota28=IOTA28, ident=IDENT,
                   possegT=np.ascontiguousarray(posseg.T), ones1=ONES1,
                   selLT=selLT, selRT=selRT, maskT=maskT)
    in_maps2 = []
    for core in range(8):
        g, i = core // 4, core % 4
        la = p["loc_attn"][g]
        in_maps2.append(dict(
            x2=np.ascontiguousarray(x_enc[2 * i:2 * i + 2, :, 16 * g:16 * g + 16]),
            xm2=xm[2 * i:2 * i + 2],
            wq=_np(la["wq"]), wk=_np(la["wk"]), wv=_np(la["wv"]), wo=_np(la["wo"]),
            bq=_np(la["bq"]), bk=_np(la["bk"]),
            bv=_np(la["bv"]).reshape(1, D), bo=_np(la["bo"]).reshape(1, D),
            wcatg=_pad_wcat(_np(p["grp_conv"][g])), **common2))
    res2 = run_bass_kernel_spmd(nc2, in_maps2, core_ids=list(range(8)))
    if _timing is not None:
        _timing.append(res2)

    out = np.zeros((B, PRED, C), np.float32)
    for core in range(8):
        g, i = core // 4, core % 4
        y = res2.results[core]["y"]  # [96, 2]
        for j in range(2):
            out[2 * i + j, :, 16 * g:16 * g + 16] = y[:, j][:, None]
    return out
